# revision 44
# baseline (speedup 1.0000x reference)
"""Trainium2 Bass kernel for Physics-Attention over an irregular mesh.

Contract: kernel(**inputs) takes the FULL inputs from setup_inputs() and
returns the FULL [4, 32768, 256] f32 output, distributing across 8 cores
internally (one (batch, half-of-N) shard per core, pairwise AllReduce on the
slice-token pooling reductions).

Structure (v3):
  pass 1 per 128-token tile (lg software-pipelined one tile ahead so the
  PE computes logits(t+1) while waiting on swn(t)):
    logits = x @ A.T          (2 matmuls, A = folded Wslice@Wx/temp)
    usw    = exp(logits)      (scalar)
    den    = row-sums per head (vector), rden = 1/den (vector)
    swn    = usw * rden        (gpsimd, bf16)
    swT    : 4 PE transposes of swn, copied to SBUF (vector)
    pool   : st_x[hg, :] += swn_chunkᵀ @ [x | 1]   (4 matmuls, shared rhs)
  stage: project st_x through Wfx at slice level, pairwise AllReduce of
    [64, 8, 65], then the slice cross-attention (per-head matmuls packed
    2-heads-wide) producing C[hg, 256] with Wout folded.
  pass 2 per tile: out = swTᵀ @ C (4 matmuls, PSUM tag-alternated), bf16 out.
"""

import sys

sys.path.insert(0, "/opt/trn_rl_repo")

import numpy as np
import ml_dtypes

import concourse.bass as bass
import concourse.mybir as mybir
import concourse.tile as tile
from concourse import bacc, bass_utils
from concourse.bass import ts

F32 = mybir.dt.float32
BF16 = mybir.dt.bfloat16
AF = mybir.ActivationFunctionType
ALU = mybir.AluOpType

B, N, DIM = 4, 32768, 256
H, D, G = 8, 64, 64
INNER = H * D  # 512
NCORES = 8
NLOC = N // 2          # 16384 tokens per core
TOK = 128              # tokens per tile
T = NLOC // TOK        # 128 tiles
KCH = DIM // 128       # 2 contraction chunks
XA = DIM + 2           # x_aug columns: 256 features, ones col, pad
EPS_SLICE = 1e-5

_CACHE = {}


def _build(attn_scale: float, res_scale: float):
    """Build the single-core SPMD program (identical on all 8 cores)."""
    nc = bacc.Bacc("TRN2", target_bir_lowering=False, debug=False,
                   enable_asserts=False, num_devices=NCORES)

    xT_d = nc.dram_tensor("xT", [DIM, NLOC], BF16, kind="ExternalInput").ap()
    xa_d = nc.dram_tensor("xa", [NLOC, XA], BF16, kind="ExternalInput").ap()
    AT_d = nc.dram_tensor("AT", [DIM, INNER], BF16, kind="ExternalInput").ap()
    WfxT_d = nc.dram_tensor("WfxT", [DIM, INNER], BF16, kind="ExternalInput").ap()
    idbf_d = nc.dram_tensor("idbf", [128, 128], BF16, kind="ExternalInput").ap()
    id32_d = nc.dram_tensor("id32", [2 * D, D], F32, kind="ExternalInput").ap()
    WqT_d = nc.dram_tensor("WqT", [D, D], F32, kind="ExternalInput").ap()
    WkT_d = nc.dram_tensor("WkT", [D, D], F32, kind="ExternalInput").ap()
    WvT_d = nc.dram_tensor("WvT", [D, D], F32, kind="ExternalInput").ap()
    WoT_d = nc.dram_tensor("WoT", [INNER, DIM], BF16, kind="ExternalInput").ap()
    out_d = nc.dram_tensor("out", [NLOC, DIM], BF16, kind="ExternalOutput").ap()

    xT_v = xT_d.rearrange("(c p) n -> p c n", p=128)    # [128, 2, NLOC]
    xa_v = xa_d.rearrange("(t two p) f -> t p two f", p=TOK, two=2)  # [T/2,128,2,XA]
    AT_v = AT_d.rearrange("(c p) n -> p c n", p=128)    # [128, 2, 512]
    WfxT_v = WfxT_d.rearrange("(c p) n -> p c n", p=128)
    WoT_v = WoT_d.rearrange("(h d) f -> d h f", d=64)   # [64, 8, 256]
    out_v = out_d.rearrange("(t p) f -> t p f", p=TOK)  # [T, 128, 256]

    with tile.TileContext(nc) as tc:
        with (
            tc.tile_pool(name="consts", bufs=1) as consts,
            tc.tile_pool(name="store", bufs=1) as store,
            tc.tile_pool(name="work", bufs=5) as work,
            tc.tile_pool(name="small", bufs=6) as small,
            tc.tile_pool(name="stage", bufs=1) as stg_pool,
            tc.tile_pool(name="psmm", bufs=2, space="PSUM") as psmm,
            tc.tile_pool(name="psacc", bufs=1, space="PSUM") as psacc,
            tc.tile_pool(name="dram", bufs=1, space="DRAM") as dram,
        ):
            # resident transposed routing weights: [128, tile, chunk, tok] bf16
            swT_store = store.tile([128, T, 4, TOK], BF16)
            # x-space slice-token accumulators, one PSUM bank per hg-chunk:
            # st_ps[c][r, 0:256] = sum_n swn[n, 128c+r] * x[n, :],
            # col 256 = snorm for that hg row.
            st_ps = [psacc.tile([128, XA], F32, name=f"st_ps{c}")
                     for c in range(4)]

            # ---- first tile data + pass-1 constants, then stage constants --
            xt2s, xa2s = {}, {}

            def dma_quad(q):
                xt4 = work.tile([128, KCH, 4 * TOK], BF16, tag="xt2")
                nc.sync.dma_start(xt4, xT_v[:, :, q * 4 * TOK:(q + 1) * 4 * TOK])
                xa4 = work.tile([128, 4, XA], BF16, tag="xa2")
                nc.sync.dma_start(
                    xa4.rearrange("p (two2 two) f -> p two2 two f", two=2),
                    xa_v[2 * q:2 * q + 2].rearrange("u p two f -> p u two f"))
                xt2s[q], xa2s[q] = xt4, xa4

            # tile-0 data + AT split across DMA queues to cut startup latency
            xt4_0 = work.tile([128, KCH, 4 * TOK], BF16, tag="xt2")
            for k in range(KCH):
                nc.sync.dma_start(xt4_0[:, k], xT_v[:, k, 0:4 * TOK])
            xa4_0 = work.tile([128, 4, XA], BF16, tag="xa2")
            for two in range(4):
                nc.sync.dma_start(xa4_0[:, two],
                                  xa_v[two // 2][:, two % 2])
            xt2s[0], xa2s[0] = xt4_0, xa4_0
            AT_sb = consts.tile([128, KCH, INNER], BF16)
            for k in range(KCH):
                nc.sync.dma_start(AT_sb[:, k], AT_v[:, k])
            idbf = consts.tile([128, 128], BF16)
            nc.sync.dma_start(idbf, idbf_d)
            dma_quad(1)
            # stage-only constants (needed only after pass 1; load overlaps it)
            WfxT_sb = consts.tile([128, KCH, INNER], BF16)
            nc.sync.dma_start(WfxT_sb, WfxT_v)
            id32 = consts.tile([128, 64], F32)
            nc.sync.dma_start(id32, id32_d)
            WqT_sb = consts.tile([64, 64], F32)
            nc.sync.dma_start(WqT_sb, WqT_d)
            WkT_sb = consts.tile([64, 64], F32)
            nc.sync.dma_start(WkT_sb, WkT_d)
            WvT_sb = consts.tile([64, 64], F32)
            nc.sync.dma_start(WvT_sb, WvT_d)
            WoT_sb = consts.tile([64, H, DIM], BF16)
            nc.sync.dma_start(WoT_sb, WoT_v)

            # ================= PASS 1 =================
            for t in range(T):
                if t % 4 == 0 and t // 4 + 2 < T // 4:
                    dma_quad(t // 4 + 2)
                xt = xt2s[t // 4][:, :, (t % 4) * TOK:(t % 4 + 1) * TOK]
                xa = xa2s[t // 4][:, t % 4, :]

                lg = psmm.tile([128, H, G], F32, tag="lg")
                for k in range(KCH):
                    nc.tensor.matmul(lg, xt[:, k, :], AT_sb[:, k, :],
                                     start=(k == 0), stop=(k == KCH - 1))

                usw = work.tile([128, H, G], BF16, tag="usw")
                nc.scalar.activation(usw, lg, AF.Exp)
                den = small.tile([128, H], BF16, tag="den")
                with nc.allow_low_precision(reason="softmax denom tolerates bf16"):
                    nc.vector.reduce_sum(den, usw, axis=mybir.AxisListType.X)
                rden = small.tile([128, H], F32, tag="rden")
                nc.vector.reciprocal(rden, den)
                swn = work.tile([128, H, G], BF16, tag="swn")
                nc.gpsimd.tensor_tensor(
                    swn, usw, rden[:, :, None].to_broadcast([128, H, G]), ALU.mult)

                swn2 = swn.rearrange("p h g -> p (h g)")
                for c in range(4):
                    nc.tensor.matmul(st_ps[c], swn2[:, ts(c, 128)], xa,
                                     start=(t == 0), stop=(t == T - 1))

                swt = psmm.tile([128, 4, TOK], BF16, tag="tp")
                for c in range(4):
                    nc.tensor.transpose(swt[:, c, :], swn2[:, ts(c, 128)], idbf)
                nc.vector.tensor_copy(swT_store[:, t, 0:3, :], swt[:, 0:3, :])
                nc.scalar.copy(swT_store[:, t, 3, :], swt[:, 3, :])

            # ===== STAGE: project st_x -> st_un per head, AllReduce, attn ====
            # evacuate accumulators (cast x-part to bf16, keep snorm f32)
            stx_bf = stg_pool.tile([128, 4, DIM], BF16)
            snorm_sb = stg_pool.tile([128, 4], BF16)
            for c in range(4):
                nc.vector.tensor_copy(stx_bf[:, c, :], st_ps[c][:, 0:DIM])
            for c in range(4):
                nc.vector.tensor_copy(snorm_sb[:, c:c + 1], st_ps[c][:, DIM:DIM + 1])

            # transpose stx_bf to [f, hg] (8 PE transposes + 2 copies)
            stxT = stg_pool.tile([128, KCH, 4, 128], BF16)
            for k in range(KCH):
                tpp = psmm.tile([128, 4, 128], BF16, tag="tp")
                for c in range(4):
                    nc.tensor.transpose(tpp[:, c, :],
                                        stx_bf[:, c, ts(k, 128)], idbf)
                nc.vector.tensor_copy(stxT[:, k], tpp)

            # st_un[g, h, d] = sum_f st_x[h g, f] * Wfx[h d, f]
            st_un = psmm.tile([64, H, D], F32, tag="lg")
            for h in range(H):
                c, r = h // 2, h % 2
                for k in range(KCH):
                    nc.tensor.matmul(
                        st_un[:, h, :],
                        stxT[:, k, c, ts(r, 64)],
                        WfxT_sb[:, k, ts(h, D)],
                        start=(k == 0), stop=(k == KCH - 1))
            stun_sb = stg_pool.tile([64, H, D], BF16)
            nc.vector.tensor_copy(stun_sb, st_un)

            # pack AllReduce buffer [64, H, D+1]: cols 0:64 st_un, col 64 snorm
            cc_in = dram.tile([64, H * (D + 1)], BF16)
            cc_out = dram.tile([64, H * (D + 1)], BF16)
            cc_v = cc_in.rearrange("g (h e) -> g h e", h=H)
            nc.sync.dma_start(cc_v[:, :, 0:D], stun_sb)
            cc_h = cc_in.rearrange("g (c r e) -> g c r e", c=4, r=2)
            for r in range(2):
                nc.sync.dma_start(cc_h[:, :, r, D], snorm_sb[ts(r, 64), :])
            nc.gpsimd.collective_compute(
                "AllReduce", ALU.add,
                replica_groups=[[0, 1], [2, 3], [4, 5], [6, 7]],
                ins=[cc_in.opt()], outs=[cc_out.opt()],
            )
            stg = stg_pool.tile([64, H, D + 1], BF16)
            nc.sync.dma_start(stg.rearrange("p h e -> p (h e)"), cc_out)

            snorm_e = stg_pool.tile([64, H], F32)
            nc.vector.tensor_scalar_add(snorm_e, stg[:, :, D], EPS_SLICE)
            rs = stg_pool.tile([64, H], F32)
            nc.vector.reciprocal(rs, snorm_e)
            st_sb = stg_pool.tile([64, H, D], F32)
            nc.vector.tensor_tensor(st_sb, stg[:, :, 0:D],
                                    rs[:, :, None].to_broadcast([64, H, D]),
                                    ALU.mult)
            kv = stg_pool.tile([64, D], F32)
            nc.vector.reduce_sum(kv, st_sb.rearrange("p h d -> p d h"),
                                 axis=mybir.AxisListType.X)

            # stT[d, h, g] via 8 PE transposes (copies batched in pairs)
            stT = stg_pool.tile([64, H, D], F32)
            for j in range(4):
                tp = psmm.tile([64, 2, 64], F32, tag="tp")
                for r in range(2):
                    nc.tensor.transpose(tp[:, r, :], st_sb[:, 2 * j + r, :],
                                        id32[0:64, :])
                nc.vector.tensor_copy(stT[:, 2 * j:2 * j + 2, :], tp)
            kvT_p = psmm.tile([64, 64], F32, tag="tp")
            nc.tensor.transpose(kvT_p, kv, id32[0:64, :])
            kvT = stg_pool.tile([64, D], F32)
            nc.vector.tensor_copy(kvT, kvT_p)

            # q = st @ WqT (per head), k/v from kv
            q_ps = psmm.tile([64, H, D], F32, tag="lg")
            for h in range(H):
                nc.tensor.matmul(q_ps[:, h, :], stT[:, h, :], WqT_sb,
                                 start=(h == 0), stop=(h == H - 1))
            k_ps = psmm.tile([64, D], F32, tag="tp")
            nc.tensor.matmul(k_ps, kvT, WkT_sb, start=True, stop=True)
            v_ps = psmm.tile([64, D], F32, tag="tp")
            nc.tensor.matmul(v_ps, kvT, WvT_sb, start=True, stop=True)
            v_sb = stg_pool.tile([64, D], F32)
            nc.vector.tensor_copy(v_sb, v_ps)

            # rnorms for q and k: squares on vector, Sqrt calls adjacent
            # (single activation-table load)
            q_sb = stg_pool.tile([64, H, D], F32)
            nc.vector.tensor_copy(q_sb, q_ps)
            k_sb = stg_pool.tile([64, D], F32)
            nc.vector.tensor_copy(k_sb, k_ps)
            qsq = stg_pool.tile([64, H, D], F32)
            nc.vector.tensor_mul(qsq, q_sb, q_sb)
            ksq = stg_pool.tile([64, D], F32)
            nc.vector.tensor_mul(ksq, k_sb, k_sb)
            qn2 = stg_pool.tile([64, H], F32)
            nc.vector.reduce_sum(qn2, qsq, axis=mybir.AxisListType.X)
            kn2 = stg_pool.tile([64, 1], F32)
            nc.vector.reduce_sum(kn2, ksq[:, None, :], axis=mybir.AxisListType.X)
            qr0 = stg_pool.tile([64, H], F32)
            nc.vector.reciprocal(qr0, qn2)
            kr0 = stg_pool.tile([64, 1], F32)
            nc.vector.reciprocal(kr0, kn2)
            qy0 = stg_pool.tile([64, H], F32)
            nc.scalar.activation(qy0, qr0, AF.Sqrt)
            ky0 = stg_pool.tile([64, 1], F32)
            nc.scalar.activation(ky0, kr0, AF.Sqrt)

            def newton(y0, n2, nh, tag):
                t1 = stg_pool.tile([64, nh], F32, name=f"t1_{tag}")
                nc.vector.tensor_mul(t1, y0, y0)
                nc.vector.tensor_mul(t1, t1, n2)
                nc.vector.tensor_scalar(t1, t1, -0.5, 1.5, ALU.mult, ALU.add)
                nc.vector.tensor_mul(t1, t1, y0)
                return t1

            rq = newton(qy0, qn2, H, "q")
            rk = newton(ky0, kn2, 1, "k")

            qn = stg_pool.tile([64, H, D], F32)
            nc.vector.tensor_tensor(qn, q_sb,
                                    rq[:, :, None].to_broadcast([64, H, D]),
                                    ALU.mult)
            kn = stg_pool.tile([64, D], F32)
            nc.vector.tensor_tensor(kn, k_sb,
                                    rk[:, 0:1].to_broadcast([64, D]), ALU.mult)

            # qnT[d, h, g] via 8 transposes (copies batched in pairs)
            qnT = stg_pool.tile([64, H, D], F32)
            for j in range(4):
                tp = psmm.tile([64, 2, 64], F32, tag="tp")
                for r in range(2):
                    nc.tensor.transpose(tp[:, r, :], qn[:, 2 * j + r, :],
                                        id32[0:64, :])
                nc.vector.tensor_copy(qnT[:, 2 * j:2 * j + 2, :], tp)
            knT_p = psmm.tile([64, 64], F32, tag="tp")
            nc.tensor.transpose(knT_p, kn, id32[0:64, :])
            knT = stg_pool.tile([64, D], F32)
            nc.vector.tensor_copy(knT, knT_p)

            # attention logits both orientations, exp, denominators
            L_ps = psmm.tile([64, H, G], F32, tag="lg")
            for h in range(H):
                nc.tensor.matmul(L_ps[:, h, :], qnT[:, h, :], knT,
                                 start=(h == 0), stop=(h == H - 1))
            e_sb = stg_pool.tile([64, H, G], F32)
            nc.scalar.activation(e_sb, L_ps, AF.Exp, scale=attn_scale)
            aden = stg_pool.tile([64, H], F32)
            nc.vector.reduce_sum(aden, e_sb, axis=mybir.AxisListType.X)
            ra = stg_pool.tile([64, H], F32)
            nc.vector.reciprocal(ra, aden)

            LT_ps = psmm.tile([64, 4, 2 * G], F32, tag="tp")
            for j in range(4):
                nc.tensor.matmul(LT_ps[:, j, :], knT,
                                 qnT[:, 2 * j:2 * j + 2, :],
                                 start=(j == 0), stop=(j == 3))
            eT_sb = stg_pool.tile([64, 4, 2 * G], F32)
            nc.scalar.activation(eT_sb, LT_ps, AF.Exp, scale=attn_scale)
            eT_v = eT_sb.rearrange("p j (r g) -> p (j r) g", r=2)

            av_ps = psmm.tile([64, H, D], F32, tag="lg")
            for h in range(H):
                nc.tensor.matmul(av_ps[:, h, :], eT_v[:, h, :], v_sb,
                                 start=(h == 0), stop=(h == H - 1))

            os_sb = stg_pool.tile([64, H, D], BF16)
            nc.vector.tensor_tensor(os_sb, av_ps,
                                    ra[:, :, None].to_broadcast([64, H, D]),
                                    ALU.mult)

            # osT[d, h, g], residual added in transposed space:
            # osT = osT_attn + res_scale * stT
            osT_at = stg_pool.tile([64, H, D], BF16)
            for j in range(4):
                tp = psmm.tile([64, 2, 64], BF16, tag="tp")
                for r in range(2):
                    nc.tensor.transpose(tp[:, r, :], os_sb[:, 2 * j + r, :],
                                        idbf[0:64, 0:64])
                nc.vector.tensor_copy(osT_at[:, 2 * j:2 * j + 2, :], tp)
            osT = stg_pool.tile([64, H, D], BF16)
            nc.vector.scalar_tensor_tensor(osT, stT, res_scale, osT_at,
                                           ALU.mult, ALU.add)

            C_sb = stg_pool.tile([128, 4, DIM], BF16)
            for j in range(4):
                C_ps = psmm.tile([128, DIM], F32, tag="tp")
                for par in range(2):
                    h = 2 * j + par
                    nc.tensor.matmul(C_ps[64 * par:64 * par + 64, :],
                                     osT[:, h, :], WoT_sb[:, h, :],
                                     start=True, stop=True)
                nc.vector.tensor_copy(C_sb[:, j, :], C_ps)

            # ================= PASS 2 =================
            out_v4 = out_d.rearrange("(u four p) f -> u p four f", p=TOK, four=4)
            for u in range(T // 4):
                ob4 = work.tile([128, 4, DIM], BF16, tag="ob")
                for i in range(4):
                    t = 4 * u + i
                    op = psmm.tile([128, DIM], F32,
                                   tag=("lg" if i % 2 == 0 else "tp"))
                    for cc in range(4):
                        nc.tensor.matmul(op, swT_store[:, t, cc, :],
                                         C_sb[:, cc, :],
                                         start=(cc == 0), stop=(cc == 3))
                    nc.vector.tensor_copy(ob4[:, i, :], op)
                if u >= T // 4 - 2:
                    # tail: spread the final transfers across queues so the
                    # last tile's write isn't serialized on one DMA ring
                    for i in range(4):
                        nc.sync.dma_start(out_v4[u][:, i, :], ob4[:, i, :])
                else:
                    nc.sync.dma_start(out_v4[u], ob4)

    nc.finalize()
    return nc


def kernel(x, Wfx, bfx, Wx, bx, Wslice, bslice, temp, Wq, Wk, Wv,
           res_scale, attn_scale, Wout, bout):
    x = np.asarray(x, dtype=np.float32)
    Wfx = np.asarray(Wfx, np.float32); bfx = np.asarray(bfx, np.float32)
    Wx = np.asarray(Wx, np.float32); bx = np.asarray(bx, np.float32)
    Wslice = np.asarray(Wslice, np.float32); bslice = np.asarray(bslice, np.float32)
    temp = np.asarray(temp, np.float32).reshape(H)
    Wq = np.asarray(Wq, np.float32); Wk = np.asarray(Wk, np.float32)
    Wv = np.asarray(Wv, np.float32)
    res_scale_f = float(np.asarray(res_scale, np.float32))
    attn = np.asarray(attn_scale, np.float32).reshape(H)
    Wout = np.asarray(Wout, np.float32); bout = np.asarray(bout, np.float32)

    assert np.all(np.abs(bfx) == 0) and np.all(np.abs(bx) == 0) \
        and np.all(np.abs(bslice) == 0), "nonzero projection biases unsupported"
    assert np.ptp(attn) == 0, "non-uniform attn_scale unsupported"
    attn_f = float(attn[0])

    # folded logits weight: logits[:, h*G+g] = x @ ((Wslice @ Wx_h)/temp_h).T
    A = np.concatenate(
        [(Wslice @ Wx[h * D:(h + 1) * D, :]) / temp[h] for h in range(H)], axis=0)
    AT = np.ascontiguousarray(A.T).astype(ml_dtypes.bfloat16)          # [256, 512]
    WfxT = np.ascontiguousarray(Wfx.T).astype(ml_dtypes.bfloat16)      # [256, 512]
    WoT = np.ascontiguousarray(Wout.T).astype(ml_dtypes.bfloat16)      # [512, 256]
    WqT = np.ascontiguousarray(Wq.T)
    WkT = np.ascontiguousarray(Wk.T) / H
    WvT = np.ascontiguousarray(Wv.T) / H
    idbf = np.eye(128, dtype=np.float32).astype(ml_dtypes.bfloat16)
    id32 = np.concatenate([np.eye(64, dtype=np.float32)] * 2, axis=0)

    key = (attn_f, res_scale_f)
    if key not in _CACHE:
        _CACHE[key] = _build(attn_f, res_scale_f)
    nc = _CACHE[key]

    in_maps = []
    for c in range(NCORES):
        b, half = c // 2, c % 2
        xs = x[b, half * NLOC:(half + 1) * NLOC, :]       # [16384, 256]
        xT = np.ascontiguousarray(xs.T.astype(ml_dtypes.bfloat16))
        xa = np.zeros((NLOC, XA), dtype=ml_dtypes.bfloat16)
        xa[:, 0:DIM] = xs.astype(ml_dtypes.bfloat16)
        xa[:, DIM] = 1.0
        in_maps.append(dict(xT=xT, xa=xa, AT=AT, WfxT=WfxT, idbf=idbf,
                            id32=id32, WqT=WqT, WkT=WkT, WvT=WvT, WoT=WoT))

    global _LAST_IN_MAPS
    _LAST_IN_MAPS = in_maps
    res = bass_utils.run_bass_kernel_spmd(nc, in_maps, core_ids=list(range(NCORES)))

    out = np.empty((B, N, DIM), np.float32)
    for c in range(NCORES):
        b, half = c // 2, c % 2
        out[b, half * NLOC:(half + 1) * NLOC, :] = \
            res.results[c]["out"].astype(np.float32)
    if np.any(bout):
        out += bout
    return out


# revision 45
# speedup vs baseline: 1.0203x; 1.0203x over previous
"""Trainium2 Bass kernel for Physics-Attention over an irregular mesh.

Contract: kernel(**inputs) takes the FULL inputs from setup_inputs() and
returns the FULL [4, 32768, 256] f32 output, distributing across 8 cores
internally (one (batch, half-of-N) shard per core, pairwise AllReduce on the
slice-token pooling reductions).

Structure (v3):
  pass 1 per 128-token tile (lg software-pipelined one tile ahead so the
  PE computes logits(t+1) while waiting on swn(t)):
    logits = x @ A.T          (2 matmuls, A = folded Wslice@Wx/temp)
    usw    = exp(logits)      (scalar)
    den    = row-sums per head (vector), rden = 1/den (vector)
    swn    = usw * rden        (gpsimd, bf16)
    swT    : 4 PE transposes of swn, copied to SBUF (vector)
    pool   : st_x[hg, :] += swn_chunkᵀ @ [x | 1]   (4 matmuls, shared rhs)
  stage: project st_x through Wfx at slice level, pairwise AllReduce of
    [64, 8, 65], then the slice cross-attention (per-head matmuls packed
    2-heads-wide) producing C[hg, 256] with Wout folded.
  pass 2 per tile: out = swTᵀ @ C (4 matmuls, PSUM tag-alternated), bf16 out.
"""

import sys

sys.path.insert(0, "/opt/trn_rl_repo")

import numpy as np
import ml_dtypes

import concourse.bass as bass
import concourse.mybir as mybir
import concourse.tile as tile
from concourse import bacc, bass_utils
from concourse.bass import ts

F32 = mybir.dt.float32
BF16 = mybir.dt.bfloat16
AF = mybir.ActivationFunctionType
ALU = mybir.AluOpType

B, N, DIM = 4, 32768, 256
H, D, G = 8, 64, 64
INNER = H * D  # 512
NCORES = 8
NLOC = N // 2          # 16384 tokens per core
TOK = 128              # tokens per tile
T = NLOC // TOK        # 128 tiles
KCH = DIM // 128       # 2 contraction chunks
XA = DIM + 2           # x_aug columns: 256 features, ones col, pad
EPS_SLICE = 1e-5

_CACHE = {}


def _build(attn_scale: float, res_scale: float):
    """Build the single-core SPMD program (identical on all 8 cores)."""
    nc = bacc.Bacc("TRN2", target_bir_lowering=False, debug=False,
                   enable_asserts=False, num_devices=NCORES)

    xT_d = nc.dram_tensor("xT", [DIM, NLOC], BF16, kind="ExternalInput").ap()
    xa_d = nc.dram_tensor("xa", [NLOC, XA], BF16, kind="ExternalInput").ap()
    AT_d = nc.dram_tensor("AT", [DIM, INNER], BF16, kind="ExternalInput").ap()
    WfxT_d = nc.dram_tensor("WfxT", [DIM, INNER], BF16, kind="ExternalInput").ap()
    idbf_d = nc.dram_tensor("idbf", [128, 128], BF16, kind="ExternalInput").ap()
    id32_d = nc.dram_tensor("id32", [2 * D, D], F32, kind="ExternalInput").ap()
    WqT_d = nc.dram_tensor("WqT", [D, D], F32, kind="ExternalInput").ap()
    WkT_d = nc.dram_tensor("WkT", [D, D], F32, kind="ExternalInput").ap()
    WvT_d = nc.dram_tensor("WvT", [D, D], F32, kind="ExternalInput").ap()
    WoT_d = nc.dram_tensor("WoT", [INNER, DIM], BF16, kind="ExternalInput").ap()
    out_d = nc.dram_tensor("out", [NLOC, DIM], BF16, kind="ExternalOutput").ap()

    xT_v = xT_d.rearrange("(c p) n -> p c n", p=128)    # [128, 2, NLOC]
    xa_v = xa_d.rearrange("(t two p) f -> t p two f", p=TOK, two=2)  # [T/2,128,2,XA]
    AT_v = AT_d.rearrange("(c p) n -> p c n", p=128)    # [128, 2, 512]
    WfxT_v = WfxT_d.rearrange("(c p) n -> p c n", p=128)
    WoT_v = WoT_d.rearrange("(h d) f -> d h f", d=64)   # [64, 8, 256]
    out_v = out_d.rearrange("(t p) f -> t p f", p=TOK)  # [T, 128, 256]

    with tile.TileContext(nc) as tc:
        with (
            tc.tile_pool(name="consts", bufs=1) as consts,
            tc.tile_pool(name="store", bufs=1) as store,
            tc.tile_pool(name="work", bufs=5) as work,
            tc.tile_pool(name="small", bufs=6) as small,
            tc.tile_pool(name="stage", bufs=1) as stg_pool,
            tc.tile_pool(name="psmm", bufs=2, space="PSUM") as psmm,
            tc.tile_pool(name="psacc", bufs=1, space="PSUM") as psacc,
            tc.tile_pool(name="dram", bufs=1, space="DRAM") as dram,
        ):
            # resident transposed routing weights: [128, tile, chunk, tok] bf16
            swT_store = store.tile([128, T, 4, TOK], BF16)
            # x-space slice-token accumulators, one PSUM bank per hg-chunk:
            # st_ps[c][r, 0:256] = sum_n swn[n, 128c+r] * x[n, :],
            # col 256 = snorm for that hg row.
            st_ps = [psacc.tile([128, XA], F32, name=f"st_ps{c}")
                     for c in range(4)]

            # ---- first tile data + pass-1 constants, then stage constants --
            xt2s, xa2s = {}, {}

            def dma_quad(q):
                xt4 = work.tile([128, KCH, 4 * TOK], BF16, tag="xt2")
                nc.sync.dma_start(xt4, xT_v[:, :, q * 4 * TOK:(q + 1) * 4 * TOK])
                xa4 = work.tile([128, 4, XA], BF16, tag="xa2")
                nc.sync.dma_start(
                    xa4.rearrange("p (two2 two) f -> p two2 two f", two=2),
                    xa_v[2 * q:2 * q + 2].rearrange("u p two f -> p u two f"))
                xt2s[q], xa2s[q] = xt4, xa4

            # tile-0 data + AT split across DMA queues to cut startup latency
            xt4_0 = work.tile([128, KCH, 4 * TOK], BF16, tag="xt2")
            for k in range(KCH):
                nc.sync.dma_start(xt4_0[:, k], xT_v[:, k, 0:4 * TOK])
            xa4_0 = work.tile([128, 4, XA], BF16, tag="xa2")
            for two in range(4):
                nc.sync.dma_start(xa4_0[:, two],
                                  xa_v[two // 2][:, two % 2])
            xt2s[0], xa2s[0] = xt4_0, xa4_0
            AT_sb = consts.tile([128, KCH, INNER], BF16)
            for k in range(KCH):
                nc.sync.dma_start(AT_sb[:, k], AT_v[:, k])
            idbf = consts.tile([128, 128], BF16)
            nc.sync.dma_start(idbf, idbf_d)
            dma_quad(1)
            # stage-only constants (needed only after pass 1; load overlaps it)
            WfxT_sb = consts.tile([128, KCH, INNER], BF16)
            nc.sync.dma_start(WfxT_sb, WfxT_v)
            id32 = consts.tile([128, 64], F32)
            nc.sync.dma_start(id32, id32_d)
            WqT_sb = consts.tile([64, 64], F32)
            nc.sync.dma_start(WqT_sb, WqT_d)
            WkT_sb = consts.tile([64, 64], F32)
            nc.sync.dma_start(WkT_sb, WkT_d)
            WvT_sb = consts.tile([64, 64], F32)
            nc.sync.dma_start(WvT_sb, WvT_d)
            WoT_sb = consts.tile([64, H, DIM], BF16)
            nc.sync.dma_start(WoT_sb, WoT_v)

            # ================= PASS 1 =================
            for t in range(T):
                if t % 4 == 0 and t // 4 + 2 < T // 4:
                    dma_quad(t // 4 + 2)
                xt = xt2s[t // 4][:, :, (t % 4) * TOK:(t % 4 + 1) * TOK]
                xa = xa2s[t // 4][:, t % 4, :]

                lg = psmm.tile([128, H, G], F32, tag="lg")
                for k in range(KCH):
                    nc.tensor.matmul(lg, xt[:, k, :], AT_sb[:, k, :],
                                     start=(k == 0), stop=(k == KCH - 1))

                usw = work.tile([128, H, G], BF16, tag="usw")
                nc.scalar.activation(usw, lg, AF.Exp)
                den = small.tile([128, H], BF16, tag="den")
                with nc.allow_low_precision(reason="softmax denom tolerates bf16"):
                    nc.vector.reduce_sum(den, usw, axis=mybir.AxisListType.X)
                rden = small.tile([128, H], F32, tag="rden")
                nc.vector.reciprocal(rden, den)
                swn = work.tile([128, H, G], BF16, tag="swn")
                nc.gpsimd.tensor_tensor(
                    swn, usw, rden[:, :, None].to_broadcast([128, H, G]), ALU.mult)

                swn2 = swn.rearrange("p h g -> p (h g)")
                for c in range(4):
                    nc.tensor.matmul(st_ps[c], swn2[:, ts(c, 128)], xa,
                                     start=(t == 0), stop=(t == T - 1))

                swt = psmm.tile([128, 4, TOK], BF16, tag="tp")
                for c in range(4):
                    nc.tensor.transpose(swt[:, c, :], swn2[:, ts(c, 128)], idbf)
                nc.vector.tensor_copy(swT_store[:, t, 0:3, :], swt[:, 0:3, :])
                nc.scalar.copy(swT_store[:, t, 3, :], swt[:, 3, :])

            # ===== STAGE: project st_x -> st_un per head, AllReduce, attn ====
            # evacuate accumulators (cast x-part to bf16, keep snorm f32)
            stx_bf = stg_pool.tile([128, 4, DIM], BF16)
            snorm_sb = stg_pool.tile([128, 4], BF16)
            for c in range(4):
                nc.vector.tensor_copy(stx_bf[:, c, :], st_ps[c][:, 0:DIM])
            for c in range(4):
                nc.vector.tensor_copy(snorm_sb[:, c:c + 1], st_ps[c][:, DIM:DIM + 1])

            # transpose stx_bf to [f, hg] (8 PE transposes + 2 copies)
            stxT = stg_pool.tile([128, KCH, 4, 128], BF16)
            for k in range(KCH):
                tpp = psmm.tile([128, 4, 128], BF16, tag="tp")
                for c in range(4):
                    nc.tensor.transpose(tpp[:, c, :],
                                        stx_bf[:, c, ts(k, 128)], idbf)
                nc.vector.tensor_copy(stxT[:, k], tpp)

            # st_un[g, h, d] = sum_f st_x[h g, f] * Wfx[h d, f]
            st_un = psmm.tile([64, H, D], F32, tag="lg")
            for h in range(H):
                c, r = h // 2, h % 2
                for k in range(KCH):
                    nc.tensor.matmul(
                        st_un[:, h, :],
                        stxT[:, k, c, ts(r, 64)],
                        WfxT_sb[:, k, ts(h, D)],
                        start=(k == 0), stop=(k == KCH - 1))
            stun_sb = stg_pool.tile([64, H, D], BF16)
            nc.vector.tensor_copy(stun_sb, st_un)

            # pack AllReduce buffer [64, H, D+1]: cols 0:64 st_un, col 64 snorm
            cc_in = dram.tile([64, H * (D + 1)], BF16)
            cc_out = dram.tile([64, H * (D + 1)], BF16)
            cc_v = cc_in.rearrange("g (h e) -> g h e", h=H)
            nc.sync.dma_start(cc_v[:, :, 0:D], stun_sb)
            cc_h = cc_in.rearrange("g (c r e) -> g c r e", c=4, r=2)
            for r in range(2):
                nc.sync.dma_start(cc_h[:, :, r, D], snorm_sb[ts(r, 64), :])
            nc.gpsimd.collective_compute(
                "AllReduce", ALU.add,
                replica_groups=[[0, 1], [2, 3], [4, 5], [6, 7]],
                ins=[cc_in.opt()], outs=[cc_out.opt()],
            )
            stg = stg_pool.tile([64, H, D + 1], BF16)
            nc.sync.dma_start(stg.rearrange("p h e -> p (h e)"), cc_out)

            snorm_e = stg_pool.tile([64, H], F32)
            nc.vector.tensor_scalar_add(snorm_e, stg[:, :, D], EPS_SLICE)
            rs = stg_pool.tile([64, H], F32)
            nc.vector.reciprocal(rs, snorm_e)
            st_sb = stg_pool.tile([64, H, D], F32)
            nc.vector.tensor_tensor(st_sb, stg[:, :, 0:D],
                                    rs[:, :, None].to_broadcast([64, H, D]),
                                    ALU.mult)
            kv = stg_pool.tile([64, D], F32)
            nc.vector.reduce_sum(kv, st_sb.rearrange("p h d -> p d h"),
                                 axis=mybir.AxisListType.X)

            # stT[d, h, g] via 8 PE transposes (copies batched in pairs)
            stT = stg_pool.tile([64, H, D], F32)
            for j in range(4):
                tp = psmm.tile([64, 2, 64], F32, tag="tp")
                for r in range(2):
                    nc.tensor.transpose(tp[:, r, :], st_sb[:, 2 * j + r, :],
                                        id32[0:64, :])
                nc.vector.tensor_copy(stT[:, 2 * j:2 * j + 2, :], tp)
            kvT_p = psmm.tile([64, 64], F32, tag="tp")
            nc.tensor.transpose(kvT_p, kv, id32[0:64, :])
            kvT = stg_pool.tile([64, D], F32)
            nc.vector.tensor_copy(kvT, kvT_p)

            # q = st @ WqT (per head), k/v from kv
            q_ps = psmm.tile([64, H, D], F32, tag="lg")
            for h in range(H):
                nc.tensor.matmul(q_ps[:, h, :], stT[:, h, :], WqT_sb,
                                 start=(h == 0), stop=(h == H - 1))
            k_ps = psmm.tile([64, D], F32, tag="tp")
            nc.tensor.matmul(k_ps, kvT, WkT_sb, start=True, stop=True)
            v_ps = psmm.tile([64, D], F32, tag="tp")
            nc.tensor.matmul(v_ps, kvT, WvT_sb, start=True, stop=True)
            v_sb = stg_pool.tile([64, D], F32)
            nc.vector.tensor_copy(v_sb, v_ps)

            # rnorms for q and k: squares on vector, Sqrt calls adjacent
            # (single activation-table load)
            q_sb = stg_pool.tile([64, H, D], F32)
            nc.vector.tensor_copy(q_sb, q_ps)
            k_sb = stg_pool.tile([64, D], F32)
            nc.vector.tensor_copy(k_sb, k_ps)
            qsq = stg_pool.tile([64, H, D], F32)
            nc.vector.tensor_mul(qsq, q_sb, q_sb)
            ksq = stg_pool.tile([64, D], F32)
            nc.vector.tensor_mul(ksq, k_sb, k_sb)
            qn2 = stg_pool.tile([64, H], F32)
            nc.vector.reduce_sum(qn2, qsq, axis=mybir.AxisListType.X)
            kn2 = stg_pool.tile([64, 1], F32)
            nc.vector.reduce_sum(kn2, ksq[:, None, :], axis=mybir.AxisListType.X)
            qr0 = stg_pool.tile([64, H], F32)
            nc.vector.reciprocal(qr0, qn2)
            kr0 = stg_pool.tile([64, 1], F32)
            nc.vector.reciprocal(kr0, kn2)
            qy0 = stg_pool.tile([64, H], F32)
            nc.scalar.activation(qy0, qr0, AF.Sqrt)
            ky0 = stg_pool.tile([64, 1], F32)
            nc.scalar.activation(ky0, kr0, AF.Sqrt)

            def newton(y0, n2, nh, tag):
                t1 = stg_pool.tile([64, nh], F32, name=f"t1_{tag}")
                nc.vector.tensor_mul(t1, y0, y0)
                nc.vector.tensor_mul(t1, t1, n2)
                nc.vector.tensor_scalar(t1, t1, -0.5, 1.5, ALU.mult, ALU.add)
                nc.vector.tensor_mul(t1, t1, y0)
                return t1

            rq = newton(qy0, qn2, H, "q")
            rk = newton(ky0, kn2, 1, "k")

            qn = stg_pool.tile([64, H, D], F32)
            nc.vector.tensor_tensor(qn, q_sb,
                                    rq[:, :, None].to_broadcast([64, H, D]),
                                    ALU.mult)
            kn = stg_pool.tile([64, D], F32)
            nc.vector.tensor_tensor(kn, k_sb,
                                    rk[:, 0:1].to_broadcast([64, D]), ALU.mult)

            # qnT[d, h, g] via 8 transposes (copies batched in pairs)
            qnT = stg_pool.tile([64, H, D], F32)
            for j in range(4):
                tp = psmm.tile([64, 2, 64], F32, tag="tp")
                for r in range(2):
                    nc.tensor.transpose(tp[:, r, :], qn[:, 2 * j + r, :],
                                        id32[0:64, :])
                nc.vector.tensor_copy(qnT[:, 2 * j:2 * j + 2, :], tp)
            knT_p = psmm.tile([64, 64], F32, tag="tp")
            nc.tensor.transpose(knT_p, kn, id32[0:64, :])
            knT = stg_pool.tile([64, D], F32)
            nc.vector.tensor_copy(knT, knT_p)

            # attention logits both orientations, exp, denominators
            L_ps = psmm.tile([64, H, G], F32, tag="lg")
            for h in range(H):
                nc.tensor.matmul(L_ps[:, h, :], qnT[:, h, :], knT,
                                 start=(h == 0), stop=(h == H - 1))
            e_sb = stg_pool.tile([64, H, G], F32)
            nc.scalar.activation(e_sb, L_ps, AF.Exp, scale=attn_scale)
            aden = stg_pool.tile([64, H], F32)
            nc.vector.reduce_sum(aden, e_sb, axis=mybir.AxisListType.X)
            ra = stg_pool.tile([64, H], F32)
            nc.vector.reciprocal(ra, aden)

            LT_ps = psmm.tile([64, 4, 2 * G], F32, tag="tp")
            for j in range(4):
                nc.tensor.matmul(LT_ps[:, j, :], knT,
                                 qnT[:, 2 * j:2 * j + 2, :],
                                 start=(j == 0), stop=(j == 3))
            eT_sb = stg_pool.tile([64, 4, 2 * G], F32)
            nc.scalar.activation(eT_sb, LT_ps, AF.Exp, scale=attn_scale)
            eT_v = eT_sb.rearrange("p j (r g) -> p (j r) g", r=2)

            av_ps = psmm.tile([64, H, D], F32, tag="lg")
            for h in range(H):
                nc.tensor.matmul(av_ps[:, h, :], eT_v[:, h, :], v_sb,
                                 start=(h == 0), stop=(h == H - 1))

            os_sb = stg_pool.tile([64, H, D], BF16)
            nc.vector.tensor_tensor(os_sb, av_ps,
                                    ra[:, :, None].to_broadcast([64, H, D]),
                                    ALU.mult)

            # osT[d, h, g], residual added in transposed space:
            # osT = osT_attn + res_scale * stT
            osT_at = stg_pool.tile([64, H, D], BF16)
            for j in range(4):
                tp = psmm.tile([64, 2, 64], BF16, tag="tp")
                for r in range(2):
                    nc.tensor.transpose(tp[:, r, :], os_sb[:, 2 * j + r, :],
                                        idbf[0:64, 0:64])
                nc.vector.tensor_copy(osT_at[:, 2 * j:2 * j + 2, :], tp)
            osT = stg_pool.tile([64, H, D], BF16)
            nc.vector.scalar_tensor_tensor(osT, stT, res_scale, osT_at,
                                           ALU.mult, ALU.add)

            C_sb = stg_pool.tile([128, 4, DIM], BF16)
            for j in range(4):
                C_ps = psmm.tile([128, DIM], F32, tag="tp")
                for par in range(2):
                    h = 2 * j + par
                    nc.tensor.matmul(C_ps[64 * par:64 * par + 64, :],
                                     osT[:, h, :], WoT_sb[:, h, :],
                                     start=True, stop=True)
                nc.vector.tensor_copy(C_sb[:, j, :], C_ps)

            # ================= PASS 2 =================
            out_v4 = out_d.rearrange("(u four p) f -> u p four f", p=TOK, four=4)
            for u in range(T // 4):
                ob4 = work.tile([128, 4, DIM], BF16, tag="ob")
                for i in range(4):
                    t = 4 * u + i
                    op = psmm.tile([128, DIM], F32,
                                   tag=("lg" if i % 2 == 0 else "tp"))
                    for cc in range(4):
                        nc.tensor.matmul(op, swT_store[:, t, cc, :],
                                         C_sb[:, cc, :],
                                         start=(cc == 0), stop=(cc == 3))
                    nc.vector.tensor_copy(ob4[:, i, :], op)
                nc.sync.dma_start(out_v4[u], ob4)

    nc.finalize()
    return nc


def kernel(x, Wfx, bfx, Wx, bx, Wslice, bslice, temp, Wq, Wk, Wv,
           res_scale, attn_scale, Wout, bout):
    x = np.asarray(x, dtype=np.float32)
    Wfx = np.asarray(Wfx, np.float32); bfx = np.asarray(bfx, np.float32)
    Wx = np.asarray(Wx, np.float32); bx = np.asarray(bx, np.float32)
    Wslice = np.asarray(Wslice, np.float32); bslice = np.asarray(bslice, np.float32)
    temp = np.asarray(temp, np.float32).reshape(H)
    Wq = np.asarray(Wq, np.float32); Wk = np.asarray(Wk, np.float32)
    Wv = np.asarray(Wv, np.float32)
    res_scale_f = float(np.asarray(res_scale, np.float32))
    attn = np.asarray(attn_scale, np.float32).reshape(H)
    Wout = np.asarray(Wout, np.float32); bout = np.asarray(bout, np.float32)

    assert np.all(np.abs(bfx) == 0) and np.all(np.abs(bx) == 0) \
        and np.all(np.abs(bslice) == 0), "nonzero projection biases unsupported"
    assert np.ptp(attn) == 0, "non-uniform attn_scale unsupported"
    attn_f = float(attn[0])

    # folded logits weight: logits[:, h*G+g] = x @ ((Wslice @ Wx_h)/temp_h).T
    A = np.concatenate(
        [(Wslice @ Wx[h * D:(h + 1) * D, :]) / temp[h] for h in range(H)], axis=0)
    AT = np.ascontiguousarray(A.T).astype(ml_dtypes.bfloat16)          # [256, 512]
    WfxT = np.ascontiguousarray(Wfx.T).astype(ml_dtypes.bfloat16)      # [256, 512]
    WoT = np.ascontiguousarray(Wout.T).astype(ml_dtypes.bfloat16)      # [512, 256]
    WqT = np.ascontiguousarray(Wq.T)
    WkT = np.ascontiguousarray(Wk.T) / H
    WvT = np.ascontiguousarray(Wv.T) / H
    idbf = np.eye(128, dtype=np.float32).astype(ml_dtypes.bfloat16)
    id32 = np.concatenate([np.eye(64, dtype=np.float32)] * 2, axis=0)

    key = (attn_f, res_scale_f)
    if key not in _CACHE:
        _CACHE[key] = _build(attn_f, res_scale_f)
    nc = _CACHE[key]

    in_maps = []
    for c in range(NCORES):
        b, half = c // 2, c % 2
        xs = x[b, half * NLOC:(half + 1) * NLOC, :]       # [16384, 256]
        xT = np.ascontiguousarray(xs.T.astype(ml_dtypes.bfloat16))
        xa = np.zeros((NLOC, XA), dtype=ml_dtypes.bfloat16)
        xa[:, 0:DIM] = xs.astype(ml_dtypes.bfloat16)
        xa[:, DIM] = 1.0
        in_maps.append(dict(xT=xT, xa=xa, AT=AT, WfxT=WfxT, idbf=idbf,
                            id32=id32, WqT=WqT, WkT=WkT, WvT=WvT, WoT=WoT))

    global _LAST_IN_MAPS
    _LAST_IN_MAPS = in_maps
    res = bass_utils.run_bass_kernel_spmd(nc, in_maps, core_ids=list(range(NCORES)))

    out = np.empty((B, N, DIM), np.float32)
    for c in range(NCORES):
        b, half = c // 2, c % 2
        out[b, half * NLOC:(half + 1) * NLOC, :] = \
            res.results[c]["out"].astype(np.float32)
    if np.any(bout):
        out += bout
    return out
